# revision 14
# baseline (speedup 1.0000x reference)
"""Phi^4 lattice action on Trainium2 (Bass/Tile), 8-core data parallel.

out[b] = sum_i [ (2 + 0.5*M_SQ)*phi^2 + LAM*phi^4 ]
         - 0.5 * sum_{i,s} phi[b,i]*phi[b,shift[s,i]]

For the canonical 64x64 periodic-lattice shift set {+x,-x,+y,-y} the kinetic
term equals -sum_i phi_i*(phi_{+x} + phi_{+y}) (shift-sum symmetry over the
torus). The host pads each state to a 65x65 halo tile (col 64 = col 0,
row 64 = row 0) in FP16, so both roll products are pure strided views.

FP16 is the key speed lever: DVE tensor_tensor runs in 2x_1p perf mode on
16-bit operands (2 elem/lane/cycle; measured 2.33us per [128,4096] op,
including the odd-offset +x view), while fp32 is stuck at 1x. The rel-err
budget (2e-2) dwarfs fp16 quantization (~1e-4 end to end).

Per 128-row batch tile:
  DVE: t = phi_{+x} + phi_{+y}            (tensor_add, 2x)
       m-accum = -sum phi*t               (scalar_tensor_tensor, 1x - the
                                           only fused multiply+reduce op
                                           this walrus build compiles)
       a-rows 0..R0 = phi^2               (tensor_mul, 2x, balance filler)
  ACT: a-rows R0..64 = phi^2              (Square, 1x)
       q-accum = sum (sqrt(LAM)*a)^2      (Square + accum_out, 1x)
Engines run ~balanced (~7.5us/tile each); DMA (~2.7us/tile fp16) hides
behind compute. Final: per-tile accum columns reduced and stored as
act[P, NTILES]; host transposes.

tensor_tensor_reduce / tensor_mask_reduce / custom-DVE ops (which could
fuse the m-product at 2x) all fail this walrus codegen ("ISA wrong
length"), so STT at 1x is the best available accumulate op.

Non-lattice shift inputs fall back to a generic path: the host computes
nsum = sum_s phi[:, shift[s]] and the device evaluates
LAM*sum phi^4 - 0.5*sum phi*nsum with fused fp32 ops.
"""

import json
import math

import numpy as np

import concourse.bass as bass
import concourse.mybir as mybir
import concourse.tile as tile
from concourse.bass_utils import run_bass_kernel_spmd

def _max_waits(opcode: str) -> int:
    # This walrus build accepts at most ONE sync wait per instruction.
    return 1


def _split_excess_waits(bir_bytes: bytes) -> bytes:
    """The container's walrus codegen rejects any instruction carrying more
    than 2 sync waits ("Too many sync wait commands"), but Tile's tail drain
    and WAR-gated DMA loads can carry 3+. Peel excess waits onto injected
    same-engine Drain instructions placed immediately before the offender."""
    bir = json.loads(bir_bytes)
    n_new = 0
    for func in bir.get("functions", []):
        for bb in func.get("blocks", []):
            insts = bb.get("instructions", [])
            out = []
            for inst in insts:
                sync = inst.get("sync_info") or {}
                waits = sync.get("on_wait") or []
                cap = _max_waits(inst["opcode"])
                if len(waits) > cap:
                    extra = waits[: len(waits) - cap]
                    keep = waits[len(waits) - cap :]
                    while extra:
                        chunk, extra = extra[:1], extra[1:]
                        out.append(
                            {
                                "debug": inst.get("debug", 0),
                                "engine": inst["engine"],
                                "ins": [],
                                "name": f"{inst['name']}-wsplit{n_new}",
                                "opcode": "Drain",
                                "outs": [],
                                "sync_info": {
                                    "on_update": [],
                                    "on_wait": chunk,
                                },
                            }
                        )
                        n_new += 1
                    sync["on_wait"] = keep
                    inst["sync_info"] = sync
                out.append(inst)
            bb["instructions"] = out
    return json.dumps(bir).encode()


def _patch_json(nc):
    orig = nc.to_json_bytes

    def patched():
        return _split_excess_waits(orig())

    nc.to_json_bytes = patched
    return nc

L = 64
N = L * L  # 4096
B = 8192
NCORES = 8
BPC = B // NCORES  # 1024 rows per core
P = 128
NTILES = BPC // P  # 8

M_SQ = -4.0
LAM = 6.975
C2 = 2.0 + 0.5 * M_SQ  # == 0.0 for the reference constants
SQRT_LAM = math.sqrt(LAM)

# rows of the phi^2 tile computed on DVE (balance filler); rest on ACT
R0 = 13

TRACE = False
LAST_EXEC_NS = None

_f32 = mybir.dt.float32
_f16 = mybir.dt.float16
_bf16 = mybir.dt.bfloat16


def _neighbours(length):
    idx = np.arange(length * length).reshape(length, length)
    shifts = [
        np.roll(idx, -1, axis=1),
        np.roll(idx, 1, axis=1),
        np.roll(idx, -1, axis=0),
        np.roll(idx, 1, axis=0),
    ]
    return np.stack([s.reshape(-1) for s in shifts], axis=0)


def _is_canonical_lattice(shift: np.ndarray) -> bool:
    if shift.shape != (4, N):
        return False
    exp = np.sort(_neighbours(L), axis=0)
    got = np.sort(shift.astype(np.int64), axis=0)
    return bool(np.array_equal(exp, got))


HP = L + 1  # 65: lattice row padded with its wrap column
NP = HP * HP  # 4225: padded tile width (row 64 = row 0 + corner)


def _build_lattice():
    nc = bass.Bass()
    phi = nc.dram_tensor("phi", [BPC, NP], _f16, kind="ExternalInput")
    # [P, NTILES] so the store is contiguous per partition line; the host
    # transposes (act[p, t] = batch row t*P + p).
    act = nc.dram_tensor("act", [P, NTILES * 2], _f32, kind="ExternalOutput")

    mult = mybir.AluOpType.mult
    Square = mybir.ActivationFunctionType.Square

    CPT = 2  # kacc columns per tile: [m, q]
    SPLIT_AT = 6  # store tiles [0, SPLIT_AT) early to hide DMA latency
    # tile-0 load chunks; chunk k must cover every padded row a band's
    # +y neighbour touches, so boundaries land at rows 17/33/49
    CHR = [0, 17, 33, 49, HP]
    with tile.TileContext(nc) as tc:
        with (
            tc.tile_pool(name="io", bufs=2) as io,
            tc.tile_pool(name="tp", bufs=2) as tp,
            tc.tile_pool(name="ap", bufs=2) as ap,
            tc.tile_pool(name="jm", bufs=2) as jmp,
            tc.tile_pool(name="jq", bufs=2) as jqp,
            tc.tile_pool(name="accs", bufs=1) as accp,
        ):
            kacc = accp.tile([P, NTILES * CPT], _f32)
            kview = kacc.rearrange("p (t c) -> p t c", c=CPT)
            for t in range(NTILES):
                rows = phi[t * P : (t + 1) * P, :]
                x = io.tile([P, NP], _f16)
                x3 = x.rearrange("p (r c) -> p r c", c=HP)
                lat = x3[:, 0:L, 0:L]

                tt = tp.tile([P, N], _f16)
                t3 = tt.rearrange("p (r c) -> p r c", c=L)
                a = ap.tile([P, N], _f16)
                a3 = a.rearrange("p (r c) -> p r c", c=L)
                jm = jmp.tile([P, N], _f16)
                jq = jqp.tile([P, N], _f16)

                if t == 0:
                    # ramp: two-chunk load with banded t / phi^2 ops so
                    # compute starts after the first half lands
                    for k in (0, 2):
                        nc.sync.dma_start(
                            out=x[:, CHR[k] * HP : CHR[k + 2] * HP],
                            in_=rows[:, CHR[k] * HP : CHR[k + 2] * HP],
                        )
                        r0, r1 = 32 * (k // 2), 32 * (k // 2 + 1)
                        # DVE: t band
                        nc.vector.tensor_add(
                            t3[:, r0:r1, :],
                            x3[:, r0:r1, 1:HP],
                            x3[:, r0 + 1 : r1 + 1, 0:L],
                        )
                        # phi^2 band (DVE below R0, ACT above)
                        if r0 < R0:
                            nc.vector.tensor_mul(
                                a3[:, r0 : min(r1, R0), :],
                                x3[:, r0 : min(r1, R0), 0:L],
                                x3[:, r0 : min(r1, R0), 0:L],
                            )
                        if r1 > R0:
                            nc.scalar.activation(
                                a3[:, max(r0, R0) : r1, :],
                                x3[:, max(r0, R0) : r1, 0:L],
                                Square,
                            )
                else:
                    nc.sync.dma_start(out=x, in_=rows)
                    # ACT: phi^2 rows R0..64
                    nc.scalar.activation(
                        a3[:, R0:L, :], x3[:, R0:L, 0:L], Square
                    )
                    # DVE: t = phi_{+x} + phi_{+y}
                    nc.vector.tensor_add(
                        t3, x3[:, 0:L, 1:HP], x3[:, 1 : L + 1, 0:L]
                    )
                    # DVE: phi^2 rows 0..R0
                    if R0:
                        nc.vector.tensor_mul(
                            a3[:, 0:R0, :], x3[:, 0:R0, 0:L], x3[:, 0:R0, 0:L]
                        )
                # DVE: m-accum = -sum phi*t
                nc.vector.scalar_tensor_tensor(
                    out=jm, in0=tt, scalar=-1.0, in1=lat,
                    op0=mult, op1=mult,
                    accum_out=kview[:, t, 0:1],
                )
                # ACT: q-accum = sum (sqrt(LAM)*a)^2
                nc.scalar.activation(
                    jq, a, Square, scale=SQRT_LAM,
                    accum_out=kview[:, t, 1:2],
                )
                if t == SPLIT_AT - 1:
                    nc.sync.dma_start(
                        out=act[:, 0 : SPLIT_AT * CPT],
                        in_=kacc[:, 0 : SPLIT_AT * CPT],
                    )

            nc.sync.dma_start(
                out=act[:, SPLIT_AT * CPT :], in_=kacc[:, SPLIT_AT * CPT :]
            )
    assert C2 == 0.0  # mass term vanishes for the reference constants
    return nc


def _build_generic():
    nc = bass.Bass()
    phi = nc.dram_tensor("phi", [BPC, N], _f32, kind="ExternalInput")
    nsum = nc.dram_tensor("nsum", [BPC, N], _f32, kind="ExternalInput")
    act = nc.dram_tensor("act", [P, NTILES * 2], _f32, kind="ExternalOutput")

    mult = mybir.AluOpType.mult
    Square = mybir.ActivationFunctionType.Square

    CPT = 2
    with tile.TileContext(nc) as tc:
        with (
            tc.tile_pool(name="io", bufs=2) as io,
            tc.tile_pool(name="sq", bufs=2) as sqp,
            tc.tile_pool(name="junk", bufs=2) as junkp,
            tc.tile_pool(name="accs", bufs=1) as accp,
        ):
            kacc = accp.tile([P, NTILES * CPT], _f32)
            kview = kacc.rearrange("p (t c) -> p t c", c=CPT)
            for t in range(NTILES):
                x = io.tile([P, N], _f32)
                nc.sync.dma_start(out=x, in_=phi[t * P : (t + 1) * P, :])
                ns = io.tile([P, N], _f32)
                nc.sync.dma_start(out=ns, in_=nsum[t * P : (t + 1) * P, :])

                a = sqp.tile([P, N], _f32)
                jact = junkp.tile([P, N], _bf16)
                nc.scalar.square(a, x)
                nc.scalar.activation(
                    jact, a, Square, scale=SQRT_LAM,
                    accum_out=kview[:, t, 1:2],
                )
                jd = junkp.tile([P, N], _bf16, tag="jd_generic")
                nc.vector.scalar_tensor_tensor(
                    out=jd, in0=ns, scalar=-0.5, in1=x,
                    op0=mult, op1=mult,
                    accum_out=kview[:, t, 0:1],
                )
            nc.sync.dma_start(out=act[:, :], in_=kacc)
    assert C2 == 0.0
    return nc


_cache = {}


def _get(generic: bool):
    if generic not in _cache:
        _cache[generic] = _patch_json(
            _build_generic() if generic else _build_lattice()
        )
    return _cache[generic]


def kernel(phi_state, shift):
    global LAST_EXEC_NS
    phi = np.ascontiguousarray(np.asarray(phi_state, dtype=np.float32))
    assert phi.shape == (B, N), phi.shape
    shift_np = np.asarray(shift)

    if _is_canonical_lattice(shift_np):
        nc = _get(False)
        lat = phi.reshape(B, L, L)
        xp = np.empty((B, HP, HP), dtype=np.float16)
        xp[:, 0:L, 0:L] = lat
        xp[:, 0:L, L] = lat[:, :, 0]
        xp[:, L, 0:L] = lat[:, 0, :]
        xp[:, L, L] = lat[:, 0, 0]
        xp = xp.reshape(B, NP)
        in_maps = [
            {"phi": xp[i * BPC : (i + 1) * BPC]} for i in range(NCORES)
        ]
    else:
        nsum = np.zeros_like(phi)
        for s in range(shift_np.shape[0]):
            nsum += phi[:, shift_np[s].astype(np.int64)]
        nc = _get(True)
        in_maps = [
            {
                "phi": phi[i * BPC : (i + 1) * BPC],
                "nsum": nsum[i * BPC : (i + 1) * BPC],
            }
            for i in range(NCORES)
        ]

    r = run_bass_kernel_spmd(
        nc, in_maps, core_ids=list(range(NCORES)), trace=TRACE
    )
    LAST_EXEC_NS = r.exec_time_ns
    out = np.concatenate(
        [
            (m["act"][:, 0::2] + m["act"][:, 1::2]).T.reshape(BPC, 1)
            for m in r.results
        ],
        axis=0,
    )
    return out.astype(np.float32)
